# revision 4
# baseline (speedup 1.0000x reference)
"""Trainium2 Bass kernel for nn_DevConv (gnn_message_passing, N=8192).

Math (reference): per node i,
  maxd2[i] = relu(max over {j: adj[i,j]>0} of ||w*(x_i-x_j)||^2)
  out[i]   = 0.5*(prev[i] + mean(W_phi)*sqrt(maxd2[i]))

Distribution: node dim sharded across 8 cores; each core owns a
[1024, 8192] slab of adjacency (the memory-bound input: 32 MiB/core).

Device pipeline per core (8 i-tiles of 128 rows, each split in 4 groups
of 2048 columns):
  SP    : stream adjacency tiles HBM->SBUF (double buffered)
  ACT   : cast adjacency int32 -> bf16 (values {0,1} exact)
  PE    : psum = (sq_j - 2*y_i.y_j) via one bf16 matmul with K=21
          3-way-bf16-split rows (fp32-exact products), then accumulates
          BIG*adj via a diag(BIG) pass-through matmul of the bf16
          adjacency (start/stop accumulation in PSUM)
  DVE   : reduce-max over each [128, 2048] psum group -> acc[:, G]
Host epilogue (O(N)): fold 4 groups + 8 tiles, add sq_i - BIG, relu,
sqrt, scale by mean(W_phi), combine with prev.

The BIG-offset mask is exact: BIG > max possible d2, so for rows with a
neighbor max_j(d2 + BIG*adj) = BIG + max_nb d2; rows with no neighbor
stay < BIG and relu clamps them to 0 (matching where(...,-inf) + max(,0)).
"""
from contextlib import ExitStack

import numpy as np
import ml_dtypes

import concourse.bacc as bacc
from concourse import mybir
from concourse.bass_utils import run_bass_kernel_spmd

BF16 = ml_dtypes.bfloat16

N = 8192
CORES = 8
ROWS = N // CORES            # 1024 rows per core
TILES = ROWS // 128          # 8 i-tiles per core
GROUP_W = 2048               # columns per psum group (4 banks)
GROUPS = N // GROUP_W        # 4 groups per tile
NG = TILES * GROUPS          # 32 groups per core
CHUNK = 512                  # matmul free dim (1 psum bank fp32)
K = 21                       # contraction rows of the d2 matmul

OFF_DIAG = 0
OFF_LHST = 128
OFF_YT = 128 + ROWS
CW = 128 + ROWS + N          # 9344 bf16 columns of packed constants

_NC = {}


def _build_nc(reps=1):
    """Build the per-core program. reps>1 replays the whole pipeline on the
    same inputs (for HW-time measurement via wall-clock deltas)."""
    if reps in _NC:
        return _NC[reps]
    nc = bacc.Bacc("TRN2", target_bir_lowering=False, debug=False, num_devices=CORES)

    adj_d = nc.declare_dram_parameter("adj", [ROWS, N], mybir.dt.int32, isOutput=False)
    consts_d = nc.declare_dram_parameter(
        "consts", [128, CW], mybir.dt.bfloat16, isOutput=False
    )
    gmax_d = nc.declare_dram_parameter("gmax", [128, NG], mybir.dt.float32, isOutput=True)

    adj_sb = [nc.alloc_sbuf_tensor(f"adjsb{i}", [128, N], mybir.dt.int32) for i in range(2)]
    bf_sb = [nc.alloc_sbuf_tensor(f"bfsb{i}", [128, N], mybir.dt.bfloat16) for i in range(2)]
    consts_sb = nc.alloc_sbuf_tensor("constsb", [128, CW], mybir.dt.bfloat16)
    acc_sb = nc.alloc_sbuf_tensor("accsb", [128, NG], mybir.dt.float32)
    ps = [nc.alloc_psum_tensor(f"ps{i}", [128, GROUP_W], mybir.dt.float32) for i in range(2)]

    with ExitStack() as es:
        block = es.enter_context(nc.Block())
        const_sem = es.enter_context(nc.semaphore("const_sem"))
        a_sems = [es.enter_context(nc.semaphore(f"a_sem{t}")) for t in range(TILES)]
        act_sem = es.enter_context(nc.semaphore("act_sem"))
        pe_sem = es.enter_context(nc.semaphore("pe_sem"))
        dve_sem = es.enter_context(nc.semaphore("dve_sem"))
        out_sem = es.enter_context(nc.semaphore("out_sem"))

        NT = TILES * reps  # global tile count across reps

        @block.sync
        def _(sp):
            sp.dma_start(out=consts_sb[:, :], in_=consts_d[:, :]).then_inc(const_sem, 16)
            for T in range(NT):
                t = T % TILES
                if T >= 2:
                    # adjacency slot T%2 is free once ACT cast tile T-2
                    sp.wait_ge(act_sem, GROUPS * (T - 2) + GROUPS)
                sp.dma_start(
                    out=adj_sb[T % 2][:, :], in_=adj_d[t * 128 : (t + 1) * 128, :]
                ).then_inc(a_sems[t], 16)
            sp.wait_ge(dve_sem, NG * reps)
            sp.dma_start(out=gmax_d[:, :], in_=acc_sb[:, :]).then_inc(out_sem, 16)
            sp.wait_ge(out_sem, 16)

        @block.scalar
        def _(act):
            for T in range(NT):
                t = T % TILES
                act.wait_ge(a_sems[t], 16 * (T // TILES + 1))
                if T >= 2:
                    # bf16 slot T%2 is free once PE consumed tile T-2
                    act.wait_ge(pe_sem, GROUPS * (T - 2) + GROUPS)
                for g in range(GROUPS):
                    sl = slice(g * GROUP_W, (g + 1) * GROUP_W)
                    act.activation(
                        out=bf_sb[T % 2][:, sl],
                        in_=adj_sb[T % 2][:, sl],
                        func=mybir.ActivationFunctionType.Copy,
                    ).then_inc(act_sem)

        @block.tensor
        def _(pe):
            pe.wait_ge(const_sem, 16)
            diag = consts_sb[0:128, OFF_DIAG : OFF_DIAG + 128]
            for T in range(NT):
                t = T % TILES
                lhsT = consts_sb[0:K, OFF_LHST + t * 128 : OFF_LHST + (t + 1) * 128]
                for g in range(GROUPS):
                    G = T * GROUPS + g
                    pe.wait_ge(act_sem, G + 1)
                    if G >= 2:
                        # psum buffer G%2 is free once DVE reduced group G-2
                        pe.wait_ge(dve_sem, G - 1)
                    base = g * GROUP_W
                    for c in range(GROUP_W // CHUNK):
                        pe.matmul(
                            ps[G % 2][:, c * CHUNK : (c + 1) * CHUNK],
                            lhsT,
                            consts_sb[
                                0:K,
                                OFF_YT + base + c * CHUNK : OFF_YT + base + (c + 1) * CHUNK,
                            ],
                            start=True,
                            stop=False,
                        )
                    mm = None
                    for c in range(GROUP_W // CHUNK):
                        mm = pe.matmul(
                            ps[G % 2][:, c * CHUNK : (c + 1) * CHUNK],
                            diag,
                            bf_sb[T % 2][:, base + c * CHUNK : base + (c + 1) * CHUNK],
                            start=False,
                            stop=True,
                        )
                    mm.then_inc(pe_sem)

        @block.vector
        def _(dve):
            for G in range(NG * reps):
                dve.wait_ge(pe_sem, G + 1)
                dve.tensor_reduce(
                    out=acc_sb[:, G % NG : G % NG + 1],
                    in_=ps[G % 2][:, :],
                    axis=mybir.AxisListType.X,
                    op=mybir.AluOpType.max,
                ).then_inc(dve_sem)

    nc.compile()
    _NC[reps] = nc
    return nc


def _split3(v):
    """Exact-ish 3-way bf16 split: v ~= h + m + l with ~2^-24 rel residual."""
    h = v.astype(BF16)
    r = v - h.astype(np.float32)
    m = r.astype(BF16)
    l = (r - m.astype(np.float32)).astype(BF16)
    return h, m, l


# (lhs_part, rhs_part) product terms of (a_h+a_m+a_l)(b_h+b_m+b_l) kept
# (dropped m*l, l*m, l*l are ~2^-24 relative).
_PAIRS = [("h", "h"), ("h", "m"), ("m", "h"), ("h", "l"), ("l", "h"), ("m", "m")]


def _build_rows(y, sq):
    """lhsT rows [21, N] (columns = node i) and rhs rows [21, N] (columns = j).
    psum[i, j] = sum_k lhsT[k, i] * rhs[k, j] = sq_j - 2 * y_i . y_j"""
    n = y.shape[0]
    bh, bm, bl = _split3(y)
    b = {"h": bh, "m": bm, "l": bl}
    sh, sm, sl = _split3(sq)
    ones = np.ones(n, dtype=BF16)

    lhs_rows, rhs_rows = [], []
    for c in range(3):
        for p1, p2 in _PAIRS:
            lhs_rows.append((-2.0 * b[p1][:, c].astype(np.float32)).astype(BF16))
            rhs_rows.append(b[p2][:, c])
    for s_part in (sh, sm, sl):
        lhs_rows.append(ones)
        rhs_rows.append(s_part)
    return np.stack(lhs_rows, axis=0), np.stack(rhs_rows, axis=0)


def _prepare(previous_inclusion_score, nodes, adjacency_matrix, W_phi, W_theta):
    prev = np.asarray(previous_inclusion_score, dtype=np.float32)
    nodes = np.asarray(nodes, dtype=np.float32)
    adj = np.ascontiguousarray(np.asarray(adjacency_matrix, dtype=np.int32))
    W_phi = np.asarray(W_phi, dtype=np.float32)
    w = np.asarray(W_theta, dtype=np.float32)[:, 0]

    y = (nodes * w[None, :]).astype(np.float32)
    sq = np.sum(y * y, axis=1, dtype=np.float32)

    # BIG: power of two strictly above any possible d2 = ||y_i - y_j||^2
    bound = 4.0 * float(sq.max()) + 8.0
    BIG = np.float32(2.0 ** int(np.ceil(np.log2(bound))))

    lhsT_full, yT = _build_rows(y, sq)  # [21, N] bf16 each
    diag = (np.eye(128, dtype=np.float32) * BIG).astype(BF16)

    in_maps = []
    for k in range(CORES):
        consts = np.zeros((128, CW), dtype=BF16)
        consts[:, OFF_DIAG : OFF_DIAG + 128] = diag
        consts[0:K, OFF_LHST : OFF_LHST + ROWS] = lhsT_full[:, k * ROWS : (k + 1) * ROWS]
        consts[0:K, OFF_YT : OFF_YT + N] = yT
        in_maps.append({"adj": adj[k * ROWS : (k + 1) * ROWS], "consts": consts})
    return in_maps, prev, sq, BIG, W_phi


def _finish(res, prev, sq, BIG, W_phi):
    m = np.empty(N, dtype=np.float32)
    for k in range(CORES):
        gm = res.results[k]["gmax"]                 # [128, 32] (p, t*4+g)
        tm = gm.reshape(128, TILES, GROUPS).max(axis=2)   # [128, 8]
        m[k * ROWS : (k + 1) * ROWS] = tm.T.reshape(ROWS)

    maxd2 = np.maximum(m + sq - BIG, 0.0)
    max_dist = np.sqrt(maxd2)
    inc_mean = (max_dist[:, None] * W_phi[None, :]).mean(axis=1).astype(np.float32)
    return ((prev + inc_mean) * 0.5).astype(np.float32)


def kernel(previous_inclusion_score, nodes, adjacency_matrix, W_phi, W_theta):
    in_maps, prev, sq, BIG, W_phi = _prepare(
        previous_inclusion_score, nodes, adjacency_matrix, W_phi, W_theta
    )
    nc = _build_nc()
    res = run_bass_kernel_spmd(nc, in_maps, list(range(CORES)))
    return _finish(res, prev, sq, BIG, W_phi)


# revision 18
# speedup vs baseline: 88.8753x; 88.8753x over previous
"""Trainium2 Bass kernel for nn_DevConv (gnn_message_passing, N=8192).

Math (reference): per node i,
  maxd2[i] = relu(max over {j: adj[i,j]>0} of ||w*(x_i-x_j)||^2)
  out[i]   = 0.5*(prev[i] + mean(W_phi)*sqrt(maxd2[i]))

Distribution: node dim sharded across 8 cores; each core owns a
[1024, 8192] slab of adjacency (the memory-bound input: 32 MiB/core).

Device pipeline per core: 9 i-tiles at row step 114 (112 for the last),
each DMA'd as a FULL [128, 8192] int32 block (128 partitions = full-rate
DMA; ~12.5% overlap overhead), split into 4 column-groups of 2048:
  SP  : stream adjacency tile blocks HBM->SBUF, double buffered
  ACT : cast adjacency int32 -> bf16 ({0,1} exact) into partitions
        0..113 of the combined rhs buffer; partitions 117..127 hold the
        constant y-rows (2-way bf16 split of -2*y and sq)
  PE  : ONE matmul per 512-chunk, K=128, constant weights per tile:
        lhsT = [BIG*I ; 0 ; y-lhs rows] so
        psum[i,j] = BIG*adj[i,j] + sq_j - 2*y_i.y_j
        (identical weights for 16 consecutive matmuls keep the PE warm
        with LDWEIGHTS hidden: ~208ns per 512-col matmul)
  DVE : reduce-max over each [M_t, 2048] psum group -> acc[:, G]
Host epilogue (O(N)): fold groups/tiles, add sq_i - BIG, relu, sqrt,
scale by mean(W_phi), combine with prev.

The BIG-offset mask is exact: BIG > max possible d2, so rows with a
neighbor give BIG + max_nb d2; rows without stay < BIG and the final
relu clamps them to 0 (matching where(...,-inf) + max(,0)).
"""
from contextlib import ExitStack

import numpy as np
import ml_dtypes

import concourse.bacc as bacc
from concourse import mybir
from concourse.bass_utils import run_bass_kernel_spmd

BF16 = ml_dtypes.bfloat16

N = 8192
CORES = 8
ROWS = N // CORES            # 1024 rows per core
STEP = 114                   # i-rows advanced per tile
TILES = 9                    # 8 x 114 + 112 = 1024
K_Y = 11                     # y contraction rows (2-way split)
Y_P0 = 117                   # partition where y-rows live (117..127)
GROUP_W = 2048               # columns per psum group (4 banks)
GROUPS = N // GROUP_W        # 4 groups per tile
NG = TILES * GROUPS          # 36 groups per core
CHUNK = 512                  # matmul free dim (1 psum bank fp32)

_NC = {}


def _tile_rows(t):
    return STEP if t < TILES - 1 else ROWS - STEP * (TILES - 1)


def _build_nc(reps=1, stage="full"):
    """Build the per-core program. reps>1 replays the whole pipeline on the
    same inputs (for HW-time measurement via wall-clock deltas).
    stage in {dma, act, pe, full, peraw}: pipeline prefix, for bisection."""
    if (reps, stage) in _NC:
        return _NC[(reps, stage)]
    nc = bacc.Bacc("TRN2", target_bir_lowering=False, debug=False, num_devices=CORES)

    adj_d = nc.declare_dram_parameter("adj", [ROWS + 128, N], mybir.dt.int32, isOutput=False)
    lhsT_d = nc.declare_dram_parameter(
        "lhsT", [128, TILES * STEP + 16], mybir.dt.bfloat16, isOutput=False
    )
    yT_d = nc.declare_dram_parameter("yT", [K_Y, N], mybir.dt.bfloat16, isOutput=False)
    gmax_d = nc.declare_dram_parameter("gmax", [128, NG], mybir.dt.float32, isOutput=True)

    adj_sb = [nc.alloc_sbuf_tensor(f"adjsb{i}", [128, N], mybir.dt.int32) for i in range(2)]
    # combined rhs: partitions 0..113 <- cast adjacency, 117..127 <- y rows
    bf_sb = [nc.alloc_sbuf_tensor(f"bfsb{i}", [128, N], mybir.dt.bfloat16) for i in range(2)]
    lhsT_sb = nc.alloc_sbuf_tensor("lhsTsb", [128, TILES * STEP + 16], mybir.dt.bfloat16)
    acc_sb = nc.alloc_sbuf_tensor("accsb", [128, NG], mybir.dt.float32)
    ps = [nc.alloc_psum_tensor(f"ps{i}", [128, GROUP_W], mybir.dt.float32) for i in range(2)]

    with ExitStack() as es:
        block = es.enter_context(nc.Block())
        const_sem = es.enter_context(nc.semaphore("const_sem"))
        a_sems = [es.enter_context(nc.semaphore(f"a_sem{t}")) for t in range(TILES)]
        act_sem = es.enter_context(nc.semaphore("act_sem"))
        pe_sem = es.enter_context(nc.semaphore("pe_sem"))
        dve_sem = es.enter_context(nc.semaphore("dve_sem"))
        out_sem = es.enter_context(nc.semaphore("out_sem"))

        NT = TILES * reps  # global tile count across reps
        has_dma = stage != "peraw"
        has_act = stage in ("act", "pe", "full")
        has_pe = stage in ("pe", "full", "peraw")
        has_dve = stage == "full"

        @block.sync
        def _(sp):
            sp.dma_start(out=lhsT_sb[:, :], in_=lhsT_d[:, :]).then_inc(const_sem, 16)
            sp.dma_start(out=bf_sb[0][Y_P0 : Y_P0 + K_Y, :], in_=yT_d[:, :]).then_inc(
                const_sem, 16
            )
            sp.dma_start(out=bf_sb[1][Y_P0 : Y_P0 + K_Y, :], in_=yT_d[:, :]).then_inc(
                const_sem, 16
            )
            if has_dma:
                for T in range(NT):
                    t = T % TILES
                    if T >= 2 and has_act:
                        # adjacency slot T%2 is free once ACT cast tile T-2
                        sp.wait_ge(act_sem, GROUPS * (T - 2) + GROUPS)
                    # always a full 128-row block (full-rate DMA); the
                    # slab is padded host-side so the last tile stays aligned
                    lo = t * STEP
                    sp.dma_start(
                        out=adj_sb[T % 2][:, :], in_=adj_d[lo : lo + 128, :]
                    ).then_inc(a_sems[t], 16)
            if has_dve:
                sp.wait_ge(dve_sem, NG * reps)
            elif has_pe:
                sp.wait_ge(pe_sem, NG * reps)
            elif has_act:
                sp.wait_ge(act_sem, NG * reps)
            else:
                for t in range(TILES):
                    sp.wait_ge(a_sems[t], 16 * reps)
            sp.dma_start(out=gmax_d[:, :], in_=acc_sb[:, :]).then_inc(out_sem, 16)
            sp.wait_ge(out_sem, 16)

        if has_act:

            @block.scalar
            def _(act):
                for T in range(NT):
                    t = T % TILES
                    mt = _tile_rows(t)
                    act.wait_ge(a_sems[t], 16 * (T // TILES + 1))
                    if T >= 2 and has_pe:
                        # bf16 slot T%2 is free once PE consumed tile T-2
                        act.wait_ge(pe_sem, GROUPS * (T - 2) + GROUPS)
                    for g in range(GROUPS):
                        sl = slice(g * GROUP_W, (g + 1) * GROUP_W)
                        # cast Y_P0 (117) rows, not mt: rows mt..116 are
                        # zero-weighted in lhsT but must be finite (0*NaN=NaN)
                        act.activation(
                            out=bf_sb[T % 2][0:Y_P0, sl],
                            in_=adj_sb[T % 2][0:Y_P0, sl],
                            func=mybir.ActivationFunctionType.Copy,
                        ).then_inc(act_sem)

        if has_pe:

            @block.tensor
            def _(pe):
                pe.wait_ge(const_sem, 48)
                for T in range(NT):
                    t = T % TILES
                    mt = _tile_rows(t)
                    lhsT = lhsT_sb[:, t * STEP : t * STEP + mt]
                    for g in range(GROUPS):
                        G = T * GROUPS + g
                        if has_act:
                            pe.wait_ge(act_sem, G + 1)
                        if G >= 2 and has_dve:
                            # psum buffer G%2 is free once DVE reduced group G-2
                            pe.wait_ge(dve_sem, G - 1)
                        base = g * GROUP_W
                        mm = None
                        for c in range(GROUP_W // CHUNK):
                            mm = pe.matmul(
                                ps[G % 2][0:mt, c * CHUNK : (c + 1) * CHUNK],
                                lhsT,
                                bf_sb[T % 2][:, base + c * CHUNK : base + (c + 1) * CHUNK],
                                start=True,
                                stop=True,
                            )
                        mm.then_inc(pe_sem)

        if has_dve:

            @block.vector
            def _(dve):
                for G in range(NG * reps):
                    t = (G // GROUPS) % TILES
                    mt = _tile_rows(t)
                    dve.wait_ge(pe_sem, G + 1)
                    dve.tensor_reduce(
                        out=acc_sb[0:mt, G % NG : G % NG + 1],
                        in_=ps[G % 2][0:mt, :],
                        axis=mybir.AxisListType.X,
                        op=mybir.AluOpType.max,
                    ).then_inc(dve_sem)

    nc.compile()
    _NC[(reps, stage)] = nc
    return nc


def _split2(v):
    """2-way bf16 split: v ~= h + l with ~2^-16 rel residual."""
    h = v.astype(BF16)
    l = (v - h.astype(np.float32)).astype(BF16)
    return h, l


def _build_rows(y, sq):
    """y-side lhs rows [11, N] (columns = node i, already * -2) and rhs
    rows [11, N] (columns = j): sum_k lhs[k,i]*rhs[k,j] = sq_j - 2 y_i.y_j
    (up to ~2^-16 relative from the dropped l*l products)."""
    n = y.shape[0]
    bh, bl = _split2(y)
    b = {"h": bh, "l": bl}
    sh, sl = _split2(sq)
    ones = np.ones(n, dtype=BF16)

    pairs = [("h", "h"), ("h", "l"), ("l", "h")]
    lhs_rows, rhs_rows = [], []
    for c in range(3):
        for p1, p2 in pairs:
            lhs_rows.append((-2.0 * b[p1][:, c].astype(np.float32)).astype(BF16))
            rhs_rows.append(b[p2][:, c])
    for s_part in (sh, sl):
        lhs_rows.append(ones)
        rhs_rows.append(s_part)
    return np.stack(lhs_rows, axis=0), np.stack(rhs_rows, axis=0)


def _prepare(previous_inclusion_score, nodes, adjacency_matrix, W_phi, W_theta):
    prev = np.asarray(previous_inclusion_score, dtype=np.float32)
    nodes = np.asarray(nodes, dtype=np.float32)
    adj = np.ascontiguousarray(np.asarray(adjacency_matrix, dtype=np.int32))
    W_phi = np.asarray(W_phi, dtype=np.float32)
    w = np.asarray(W_theta, dtype=np.float32)[:, 0]

    y = (nodes * w[None, :]).astype(np.float32)
    sq = np.sum(y * y, axis=1, dtype=np.float32)

    # BIG: power of two strictly above any possible d2 = ||y_i - y_j||^2
    bound = 4.0 * float(sq.max()) + 8.0
    BIG = np.float32(2.0 ** int(np.ceil(np.log2(bound))))

    ylhs, yT = _build_rows(y, sq)  # [11, N] bf16 each
    eye = np.eye(128, dtype=np.float32) * BIG

    # per-core slab views padded to ROWS+128 rows so every tile DMA is a
    # full-rate [128, N] block; pad rows are ignored by the compute
    pad_last = np.concatenate([adj[(CORES - 1) * ROWS :], adj[:128]], axis=0)
    in_maps = []
    for k in range(CORES):
        adj_k = adj[k * ROWS : k * ROWS + ROWS + 128] if k < CORES - 1 else pad_last
        # lhsT_all [128, 9*114+16]: per tile t at column offset t*STEP:
        #   rows 0..mt-1   = BIG * I[:, :mt]
        #   rows mt..116   = 0
        #   rows 117..127  = y-lhs rows for this tile's nodes
        lhsT_all = np.zeros((128, TILES * STEP + 16), dtype=BF16)
        for t in range(TILES):
            mt = _tile_rows(t)
            cols = slice(t * STEP, t * STEP + mt)
            lhsT_all[0:mt, cols] = eye[0:mt, 0:mt].astype(BF16)
            node_lo = k * ROWS + t * STEP
            lhsT_all[Y_P0:128, cols] = ylhs[:, node_lo : node_lo + mt]
        in_maps.append({"adj": adj_k, "lhsT": lhsT_all, "yT": yT})
    return in_maps, prev, sq, BIG, W_phi


def _finish(res, prev, sq, BIG, W_phi):
    m = np.empty(N, dtype=np.float32)
    for k in range(CORES):
        gm = res.results[k]["gmax"]                      # [128, 36] (p, t*4+g)
        tm = gm.reshape(128, TILES, GROUPS).max(axis=2)  # [128, 9]
        for t in range(TILES):
            mt = _tile_rows(t)
            lo = k * ROWS + t * STEP
            m[lo : lo + mt] = tm[0:mt, t]

    maxd2 = np.maximum(m + sq - BIG, 0.0)
    max_dist = np.sqrt(maxd2)
    inc_mean = (max_dist[:, None] * W_phi[None, :]).mean(axis=1).astype(np.float32)
    return ((prev + inc_mean) * 0.5).astype(np.float32)


def kernel(previous_inclusion_score, nodes, adjacency_matrix, W_phi, W_theta):
    in_maps, prev, sq, BIG, W_phi = _prepare(
        previous_inclusion_score, nodes, adjacency_matrix, W_phi, W_theta
    )
    nc = _build_nc()
    res = run_bass_kernel_spmd(nc, in_maps, list(range(CORES)))
    return _finish(res, prev, sq, BIG, W_phi)
